# revision 41
# baseline (speedup 1.0000x reference)
"""DenseAtt GNN message-passing kernel for Trainium2 (8 NeuronCores).

Computes out = adj * sigmoid(s_left[:, None] + s_right[None, :] + b)
with s_left = x @ W[:F], s_right = x @ W[F:], for x [N, F], adj [N, N].

Sharding: 1D row partition of adj / out across the 8 cores (1024 rows each).
Each core computes the s_left / s_right scores for its own 1024 rows on the
TensorEngine (transpose + matmul), AllGathers the 8 s_right shards to the
full 8192-vector, and replicates it down all 128 partitions with K=1
ones-matmuls. The streaming loop then reads each adj tile once: ACT applies
sigmoid with the per-row s_left as the activation bias, DVE multiplies by
adj, and DMA streams tiles in (HWDGE/sync) and out (SWDGE/gpsimd — separate
queues avoid head-of-line blocking). Memory-bound at ~64 MB HBM traffic per
core (~200 us at ~360 GB/s/core).
"""

import sys

import numpy as np

sys.path.insert(0, "/opt/trn_rl_repo")

N = 8192
F = 128
NCORES = 8
RPC = N // NCORES  # rows per core: 1024
P = 128
NBLK = RPC // P  # row blocks per core: 8
CCH = 2048  # streamed column chunk
NCCH = N // CCH
XTILES = N // P  # 64 x row-tiles

_nc = None
MAIN_RB = None  # debug knob: restrict streamed row blocks
STREAM_REPEAT = 1  # debug knob: repeat the streaming loop (perf timing)
ADJ_BUFS = 12
ATT_BUFS = 4
USE_CC = True  # AllGather s_right across cores instead of per-core full-x read
OUT_ENGINE = "gpsimd"  # SWDGE outs dodge the SP HWDGE FIFO; "sync" to A/B


def _build():
    from contextlib import ExitStack

    import concourse.tile as tile
    from concourse import bacc, mybir
    from concourse.masks import make_identity

    f32 = mybir.dt.float32

    nc = bacc.Bacc(
        "TRN2",
        target_bir_lowering=False,
        debug=False,
        enable_asserts=True,
        num_devices=NCORES,
    )

    adj = nc.dram_tensor("adj", [RPC, N], f32, kind="ExternalInput").ap()
    x = None if USE_CC else nc.dram_tensor("x", [N, F], f32, kind="ExternalInput").ap()
    xr = nc.dram_tensor("xr", [RPC, F], f32, kind="ExternalInput").ap()
    w2 = nc.dram_tensor("w2", [F, 2], f32, kind="ExternalInput").ap()
    brep = nc.dram_tensor("brep", [P, 1], f32, kind="ExternalInput").ap()
    out = nc.dram_tensor("out", [RPC, N], f32, kind="ExternalOutput").ap()

    GRP = 512 // P  # transposes grouped 4-per-PSUM-bank

    with tile.TileContext(nc) as tc, ExitStack() as ctx:
        # All pools live for the whole program so main-loop SBUF slots never
        # alias setup slots (aliasing serializes the first adj loads behind
        # all setup compute).
        const_pool = ctx.enter_context(tc.tile_pool(name="const", bufs=1))
        srr_pool = ctx.enter_context(tc.tile_pool(name="srr", bufs=1))
        xbuf_pool = ctx.enter_context(tc.tile_pool(name="xbuf", bufs=1))
        adj_pool = ctx.enter_context(tc.tile_pool(name="adj", bufs=ADJ_BUFS))
        att_pool = ctx.enter_context(tc.tile_pool(name="att", bufs=ATT_BUFS))
        tp_pool = ctx.enter_context(tc.tile_pool(name="tp", bufs=3, space="PSUM"))
        sp_pool = ctx.enter_context(tc.tile_pool(name="sp", bufs=2, space="PSUM"))
        slp_pool = ctx.enter_context(tc.tile_pool(name="slp", bufs=1, space="PSUM"))

        # xr first: the s_left transposes are at the head of PE's stream,
        # so their input must land first
        xr_nat = xbuf_pool.tile([P, RPC], f32)
        nc.sync.dma_start(
            xr_nat[:].rearrange("p (c f) -> p c f", f=F),
            xr.rearrange("(c p) f -> p c f", p=P),
        )
        # x in natural layout: chunk ch holds x rows [ch*1024, (ch+1)*1024) as
        # [p, c*F + f] = x[ch*1024 + c*P + p, f]. Separate tiles per chunk so
        # transposes start as soon as their chunk lands (deps are per-tile).
        x_chunks = []
        if not USE_CC:
            XCH = N // 8  # 1024 columns per chunk tile
            for ch in range(8):
                xc = xbuf_pool.tile([P, XCH], f32, tag=f"xc{ch}")
                nc.sync.dma_start(
                    xc[:].rearrange("p (c f) -> p c f", f=F),
                    x[ch * XCH : (ch + 1) * XCH].rearrange("(c p) f -> p c f", p=P),
                )
                x_chunks.append(xc)
        # constants packed into one tile (each tile pads to 4KB/partition;
        # packing frees ~16KB/partition for deeper adj buffering)
        cst = const_pool.tile([P, 272], f32)
        ident = cst[:, 0:128]
        ones = cst[:, 128:256]
        w2_sb = cst[:, 256:258]
        b_sb = cst[:, 258:259]
        sl_sb = cst[:, 260:268]  # s_left + b, block b in col b
        nc.sync.dma_start(w2_sb, w2)
        nc.sync.dma_start(b_sb, brep)
        make_identity(nc, ident)
        nc.vector.memset(ones, 1.0)

        out_eng = nc.gpsimd if OUT_ENGINE == "gpsimd" else nc.sync
        srr = srr_pool.tile([P, N], f32)  # s_right replicated on all partitions

        # PE clock warmup: dummy transposes so the s_loc matmuls hit 2.4GHz
        warm = tp_pool.tile([P, 512], f32, tag="tp")
        for i in range(GRP):
            nc.tensor.transpose(warm[:, i * P : (i + 1) * P], ones[:], ident[:])

        # s_left(+b): transpose xr chunks (grouped), matmul with w_left col
        xt8 = xbuf_pool.tile([P, RPC], f32)
        for g in range(NBLK // GRP):
            tp = tp_pool.tile([P, 512], f32, tag="tp")
            for i in range(GRP):
                c = g * GRP + i
                nc.tensor.transpose(
                    tp[:, i * P : (i + 1) * P],
                    xr_nat[:, c * P : (c + 1) * P],
                    ident[:],
                )
            nc.vector.tensor_copy(xt8[:, g * 512 : (g + 1) * 512], tp[:])
        slp = slp_pool.tile([P, NBLK], f32)
        for rb in range(NBLK):
            nc.tensor.matmul(
                slp[:, rb : rb + 1], xt8[:, rb * P : (rb + 1) * P], w2_sb[:, 0:1]
            )
        nc.vector.tensor_scalar_add(sl_sb[:], slp[:], b_sb[:])

        if USE_CC:
            # s_right shard: this core's 1024 scores from xt8 (= xr^T),
            # AllGather to the full 8192, then replicate down partitions
            # via K=1 ones-matmuls.
            dram_pool = ctx.enter_context(tc.tile_pool(name="ccd", bufs=1, space="DRAM"))
            srp_pool = ctx.enter_context(tc.tile_pool(name="srp", bufs=2, space="PSUM"))
            in_b = dram_pool.tile([1, RPC], f32)
            out_b = dram_pool.tile([NCORES, RPC], f32)
            s_loc = const_pool.tile([1, RPC], f32)
            for i in range(RPC // 512):
                srp = srp_pool.tile([1, 512], f32, tag="srp")
                nc.tensor.matmul(
                    srp[:], w2_sb[:, 1:2], xt8[:, i * 512 : (i + 1) * 512]
                )
                nc.vector.tensor_copy(s_loc[:, i * 512 : (i + 1) * 512], srp[:])
            nc.sync.dma_start(in_b[:], s_loc[:])
            nc.gpsimd.collective_compute(
                "AllGather",
                mybir.AluOpType.bypass,
                replica_groups=[list(range(NCORES))],
                ins=[in_b.opt()],
                outs=[out_b.opt()],
            )
            sr_free = const_pool.tile([1, N], f32)
            nc.sync.dma_start(sr_free[:], out_b[:].rearrange("c j -> (c j)")[None, :])
            # replication chunks interleaved with row-block 0's stream
            # tiles so the pipeline primes with minimum latency
            for cc in range(NCCH):
                for i in range(cc * (CCH // 512), (cc + 1) * (CCH // 512)):
                    sp = sp_pool.tile([P, 512], f32, tag="sp")
                    nc.tensor.matmul(
                        sp[:], ones[0:1, :], sr_free[:, i * 512 : (i + 1) * 512]
                    )
                    nc.any.tensor_copy(out=srr[:, i * 512 : (i + 1) * 512], in_=sp[:])
                cols = slice(cc * CCH, (cc + 1) * CCH)
                adj_t = adj_pool.tile([P, CCH], f32, tag="adj")
                nc.sync.dma_start(adj_t[:], adj[0:P, cols])
                att_t = att_pool.tile([P, CCH], f32, tag="att")
                nc.scalar.activation(
                    att_t[:],
                    srr[:, cols],
                    mybir.ActivationFunctionType.Sigmoid,
                    bias=sl_sb[:, 0:1],
                )
                nc.vector.tensor_mul(att_t[:], att_t[:], adj_t[:])
                out_eng.dma_start(out[0:P, cols], att_t[:])

        # xtw[f, j] = x[j, f] * w_right[f]: PE transpose groups of 4 into one
        # PSUM bank, then one ACT per-partition-scaled copy back in place
        # over the x chunk (ACT is otherwise idle during setup). Then the
        # ones-matmul sums over f with the result replicated down all 128
        # output partitions: srr chunk = s_right broadcast.
        for g in range(0 if USE_CC else XTILES // GRP):
            xc = x_chunks[g // 2]
            off = (g % 2) * 512
            tp = tp_pool.tile([P, 512], f32, tag="tp")
            for i in range(GRP):
                nc.tensor.transpose(
                    tp[:, i * P : (i + 1) * P],
                    xc[:, off + i * P : off + (i + 1) * P],
                    ident[:],
                )
            nc.scalar.mul(xc[:, off : off + 512], tp[:], w2_sb[:, 1:2])
            sp = sp_pool.tile([P, 512], f32, tag="sp")
            nc.tensor.matmul(sp[:], ones[:], xc[:, off : off + 512])
            nc.vector.tensor_copy(srr[:, g * 512 : (g + 1) * 512], sp[:])

        # steady state: stream adj, apply sigmoid(srr + s_left) and multiply
        # (row-block 0 already emitted above in CC mode)
        nblk_main = MAIN_RB if MAIN_RB is not None else NBLK
        for _rep in range(STREAM_REPEAT):
          for rb in range((1 if USE_CC and _rep == 0 else 0), nblk_main):
            rows = slice(rb * P, (rb + 1) * P)
            for cc in range(NCCH):
                cols = slice(cc * CCH, (cc + 1) * CCH)
                adj_t = adj_pool.tile([P, CCH], f32, tag="adj")
                nc.sync.dma_start(adj_t[:], adj[rows, cols])
                att_t = att_pool.tile([P, CCH], f32, tag="att")
                nc.scalar.activation(
                    att_t[:],
                    srr[:, cols],
                    mybir.ActivationFunctionType.Sigmoid,
                    bias=sl_sb[:, rb : rb + 1],
                )
                nc.vector.tensor_mul(att_t[:], att_t[:], adj_t[:])
                out_eng.dma_start(out[rows, cols], att_t[:])

    nc.compile()
    return nc


def kernel(x, adj, W, b):
    global _nc, USE_CC
    x = np.ascontiguousarray(np.asarray(x, dtype=np.float32))
    adj = np.asarray(adj, dtype=np.float32)
    W = np.asarray(W, dtype=np.float32).reshape(2 * F)
    b = np.float32(np.asarray(b).reshape(()))

    if _nc is None:
        _nc = _build()

    w2_np = np.ascontiguousarray(np.stack([W[:F], W[F:]], axis=1))
    brep_np = np.full((P, 1), b, dtype=np.float32)

    in_maps = []
    for k in range(NCORES):
        rows = slice(k * RPC, (k + 1) * RPC)
        im = {
            "adj": np.ascontiguousarray(adj[rows]),
            "xr": np.ascontiguousarray(x[rows]),
            "w2": w2_np,
            "brep": brep_np,
        }
        if not USE_CC:
            im["x"] = x
        in_maps.append(im)

    import time

    from concourse.bass_utils import run_bass_kernel_spmd

    res = None
    for attempt in range(4):
        try:
            res = run_bass_kernel_spmd(_nc, in_maps, core_ids=list(range(NCORES)))
            break
        except Exception:
            # transient NRT_EXEC_UNIT_UNRECOVERABLE wedges clear after a
            # short wait; retry before giving up
            if attempt == 3:
                if not USE_CC:
                    raise
                # last resort: rebuild without the cross-core AllGather
                # (each core re-reads the full x instead)
                USE_CC = False
                _nc = _build()
                im2 = [dict(m, x=x) for m in in_maps]
                time.sleep(40)
                res = run_bass_kernel_spmd(
                    _nc, im2, core_ids=list(range(NCORES))
                )
                break
            time.sleep(40 * (attempt + 1))
    return np.concatenate([r["out"] for r in res.results], axis=0)
